# revision 69
# baseline (speedup 1.0000x reference)
"""Multi-head causal attention (B=2, T=2048, D=1024, H=16) on 8 TRN2
NeuronCores: data parallel over batch x tensor parallel over head groups
(4 heads per core). Each core computes its group's Q/K/V projections,
causal attention, and a partial output projection; the host sums the 4
partials per batch element.

v2: bf16 operands end to end; paired-head S matmuls via PE row tiling
(two K=64 matmuls run concurrently in row groups 0/1); one fused exp per
head pair; causal masking on GpSimd; softmax reciprocals as exp(-ln d)
on Scalar; divisions deferred so their PE broadcasts never wait; all
output-projection work held as PE filler for the last (largest) q-block
where the exp stream saturates Scalar; coalesced input DMAs and PE
warm-up matmuls to bridge the NEFF preamble + first transfers; deep
Z-staging ring so output casts never wait on DMA completions, with
drain-tail PSUM tiles alternated into the freed attention banks.

Self-contained: builds the Bass/Tile kernel, runs it via
run_bass_kernel_spmd on cores 0-7, gathers on host.
"""
import numpy as np
import ml_dtypes

import concourse.bass as bass
import concourse.mybir as mybir
import concourse.tile as tile
from concourse.bass_utils import run_bass_kernel_spmd
from concourse.masks import make_identity, make_upper_triangular

P = 128
B, T, D = 2, 2048, 1024
H_LOCAL = 4          # heads per core
HD = 64              # head dim
F = H_LOCAL * HD     # 256 features per group
KO = D // P          # 8 contraction subtiles
NT = 512             # matmul moving width / PSUM bank
QJ = T // NT         # 4 q column tiles
KT = T // P          # 16 k row tiles
N_CORES = 8
LAG = 4              # S-matmul lookahead over P@V accumulation
N_WARM = 12          # PE warm-up matmuls (HAM un-throttle) during DMA wait

f32 = mybir.dt.float32
f32r = mybir.dt.float32r
bf16 = mybir.dt.bfloat16

_uid = [0]


def _legalize_single_wait(nc):
    # This walrus build accepts only ONE sem wait per instruction; hoist
    # extra waits onto single-wait NoOps placed just before the instruction.
    for fn in nc.m.functions:
        for bb in fn.blocks:
            new_list = []
            changed = False
            for inst in bb.instructions:
                si = inst.sync_info
                if si is not None and len(si.on_wait) > 1:
                    waits = list(si.on_wait)
                    for w in waits[:-1]:
                        _uid[0] += 1
                        new_list.append(mybir.InstNoOp(
                            name=f"I-waitsplit-{_uid[0]}",
                            engine=inst.engine,
                            sync_info=mybir.SyncInfo(on_wait=[w], on_update=[]),
                        ))
                    inst.sync_info = mybir.SyncInfo(
                        on_wait=[waits[-1]], on_update=list(si.on_update))
                    changed = True
                new_list.append(inst)
            if changed:
                bb.instructions.clear()
                bb.instructions.extend(new_list)


def build_nc():
    nc = bass.Bass(trn_type="TRN2", target_bir_lowering=False, debug=False,
                   num_devices=N_CORES)
    xT = nc.dram_tensor("xT", [D, T], bf16, kind="ExternalInput").ap()
    WqT = nc.dram_tensor("WqT", [D, F], bf16, kind="ExternalInput").ap()
    WkT = nc.dram_tensor("WkT", [D, F], bf16, kind="ExternalInput").ap()
    WvT = nc.dram_tensor("WvT", [D, F], bf16, kind="ExternalInput").ap()
    WoT = nc.dram_tensor("WoT", [F, D], bf16, kind="ExternalInput").ap()
    Z = nc.dram_tensor("Z", [T, D], bf16, kind="ExternalOutput").ap()

    xTr = xT.rearrange("(ko p) t -> p ko t", p=P)
    w_r = {
        "q": WqT.rearrange("(ko p) f -> p ko f", p=P),
        "k": WkT.rearrange("(ko p) f -> p ko f", p=P),
        "v": WvT.rearrange("(ko p) f -> p ko f", p=P),
    }

    with tile.TileContext(nc) as tc:
        with (
            tc.tile_pool(name="cw", bufs=1) as cw,
            tc.tile_pool(name="sb1", bufs=1) as sb1,
            tc.tile_pool(name="tp", bufs=4) as tp,
            tc.tile_pool(name="psS", bufs=2, space="PSUM") as psS,
            tc.tile_pool(name="psW", bufs=2, space="PSUM") as psW,
            tc.tile_pool(name="psO", bufs=2, space="PSUM") as psO,
        ):
            # ---- PE warm-up: matmuls on a zeroed tile while DMAs land ----
            zero512 = cw.tile([P, NT], bf16, tag="zero", name="zero512")
            nc.vector.memset(zero512[:], 0.0)
            for r in range(N_WARM // 4):
                for b in range(2):
                    wps = psS.tile([P, 2, NT], f32, tag="s2",
                                   name=f"warm{r}_{b}")
                    for half in range(2):
                        nc.tensor.matmul(wps[:, half], zero512[:, 0:P],
                                         zero512[:], start=True, stop=True)

            # ---- persistent constants / staging ----
            w_sb = {}
            for name in ("q", "k", "v"):
                w_sb[name] = sb1.tile([P, KO, F], bf16, tag=f"w{name}",
                                      name=f"w{name}")
            xt = sb1.tile([P, KO, T], bf16, tag="xt", name="xt")
            # issue order: earliest-needed first (wq+xt@qj0 gate the 1st
            # matmul).  Few LARGE transfers: each dma_start trigger costs
            # ~640ns on the issuing engine, so per-(ko) DMAs serialize the
            # whole input stream behind ~40 triggers.
            nc.sync.dma_start(w_sb["q"][:, 0:2], w_r["q"][:, 0:2])
            nc.sync.dma_start(xt[:, 0:2, 0:NT], xTr[:, 0:2, 0:NT])
            nc.sync.dma_start(w_sb["q"][:, 2:4], w_r["q"][:, 2:4])
            nc.sync.dma_start(xt[:, 2:4, 0:NT], xTr[:, 2:4, 0:NT])
            nc.sync.dma_start(w_sb["q"][:, 4:8], w_r["q"][:, 4:8])
            nc.sync.dma_start(xt[:, 4:8, 0:NT], xTr[:, 4:8, 0:NT])
            nc.sync.dma_start(w_sb["k"][:, 0:4], w_r["k"][:, 0:4])
            nc.sync.dma_start(w_sb["k"][:, 4:8], w_r["k"][:, 4:8])
            nc.sync.dma_start(w_sb["v"][:, 0:4], w_r["v"][:, 0:4])
            nc.sync.dma_start(w_sb["v"][:, 4:8], w_r["v"][:, 4:8])
            for qj in range(1, QJ):
                nc.sync.dma_start(xt[:, :, qj * NT:(qj + 1) * NT],
                                  xTr[:, :, qj * NT:(qj + 1) * NT])

            wo = cw.tile([P, F // P, D], bf16, tag="wo", name="wo")
            nc.gpsimd.dma_start(wo[:], WoT.rearrange("(fo p) d -> p fo d", p=P))
            # allowed[k_row, q_col] = q >= k (upper-triangular incl. diagonal)
            tri = cw.tile([P, P], bf16, tag="tri", name="tri")
            make_upper_triangular(nc, tri[:], val=1.0, diag=True)
            ident = cw.tile([P, P], bf16, tag="ident", name="ident")
            make_identity(nc, ident[:])
            ones_r = cw.tile([1, HD], bf16, tag="ones", name="ones")
            nc.gpsimd.memset(ones_r[:], 1.0)

            # Q/K packed 2 heads per 128 rows: rows 0:64 head 2fs, 64:128
            # head 2fs+1.  The S matmuls contract K=64 per head; the pair
            # runs concurrently in PE row groups (tile_position (0,0)/(64,0)
            # auto-derived from base partitions).
            qt = cw.tile([P, F // P, T], bf16, tag="qt", name="qt")
            kt = cw.tile([P, F // P, T], bf16, tag="kt", name="kt")

            # V with a ones column per head: [k-token, kt, head, 0:64]=V^T,
            # [..., 64]=1 (gives softmax denominators for free in P@V)
            vaug = cw.tile([P, KT, H_LOCAL, HD + 1], bf16, tag="vaug",
                           name="vaug")
            nc.gpsimd.memset(vaug[:, :, :, HD:HD + 1], 1.0)

            ot = cw.tile([P, F // P, T], bf16, tag="ot", name="ot")

            def phase1_units(qj):
                # filler units (~8 PE matmuls each) spliced into attention
                # so the PE stream never drains
                sl = slice(qj * NT, (qj + 1) * NT)
                vt = tp.tile([P, F // P, NT], bf16, tag="vt", bufs=2,
                             name=f"vt{qj}")
                units = []

                def proj(name, fs):
                    def emit():
                        ps = psW.tile([P, NT], f32, tag="w",
                                      name=f"ps_{name}{fs}_{qj}")
                        for ko in range(KO):
                            nc.tensor.matmul(
                                ps[:], w_sb[name][:, ko, fs * P:(fs + 1) * P],
                                xt[:, ko, sl],
                                start=(ko == 0), stop=(ko == KO - 1))
                        if name == "q":
                            nc.vector.tensor_copy(qt[:, fs, sl], ps[:])
                        elif name == "k":
                            nc.vector.tensor_copy(kt[:, fs, sl], ps[:])
                        else:
                            nc.vector.tensor_copy(vt[:, fs, :], ps[:])
                    return emit

                def vtrans(fs):
                    def emit():
                        pst = psW.tile([P, 4, P], bf16, tag="w",
                                       name=f"pvt{fs}_{qj}")
                        for j in range(4):
                            nc.tensor.transpose(
                                pst[:, j], vt[:, fs, j * P:(j + 1) * P],
                                ident[:])
                        nc.vector.tensor_copy(
                            vaug[:, 4 * qj:4 * qj + 4, 2 * fs:2 * fs + 2,
                                 0:HD],
                            pst[:].rearrange("p j (a b) -> p j a b", a=2))
                    return emit

                for name in ("q", "k", "v"):
                    for fs in range(F // P):
                        units.append(proj(name, fs))
                for fs in range(F // P):
                    units.append(vtrans(fs))
                return units

            # after the last attention pair the 4 psS banks are free:
            # alternating tail pz tiles between psW and psS doubles the
            # effective ring depth, so the drain-tail phase4 chain is not
            # serialized on each cast two slots back
            tailpz = [False, 0]
            # while the final division's lane-starved Ln/Exp reciprocals
            # drain on Scalar, filler casts must not queue ahead of them
            resv_dve = [False]

            def phase4_units(qj, tail=False):
                units = []

                def unit(qt_i, dt):
                    def emit():
                        tailpz[1] += 1
                        if tailpz[0] and tailpz[1] % 2 == 0:
                            pz = psS.tile([P, NT], f32, tag="s2",
                                          name=f"pz{qt_i}_{dt}")
                        else:
                            pz = psW.tile([P, NT], f32, tag="w",
                                          name=f"pz{qt_i}_{dt}")
                        for fs in range(F // P):
                            nc.tensor.matmul(
                                pz[:], ot[:, fs, qt_i * P:(qt_i + 1) * P],
                                wo[:, fs, dt * NT:(dt + 1) * NT],
                                start=(fs == 0), stop=(fs == F // P - 1))
                        zs = tp.tile([P, NT], bf16, tag="z", bufs=8,
                                     name=f"zs{qt_i}_{dt}")
                        if dt == 1 and not resv_dve[0]:
                            # alternate the PSUM->SBUF casts across both
                            # engines: halves the psW WAR chain latency;
                            # Scalar has enough bubbles even while the exp
                            # stream runs, and is idle in the drain tail
                            nc.scalar.activation(
                                zs[:], pz[:],
                                mybir.ActivationFunctionType.Copy)
                        else:
                            nc.vector.tensor_copy(zs[:], pz[:])
                        # alternate trigger queues so the output stream is
                        # not serialized behind one engine's ~640ns triggers
                        # (in the tail, keep everything on sync so gpsimd's
                        # software DMA queue can drain early)
                        if tail:
                            eng = nc.sync if (qt_i + dt) % 2 == 0 \
                                else nc.scalar
                        else:
                            eng = nc.sync if (qt_i + dt) % 2 == 0 \
                                else nc.gpsimd
                        eng.dma_start(
                            Z[qt_i * P:(qt_i + 1) * P,
                              dt * NT:(dt + 1) * NT], zs[:])
                    return emit

                for qt_i in range(4 * qj, 4 * qj + 4):
                    for dt in range(D // NT):
                        units.append(unit(qt_i, dt))
                return units

            def attn_pair(p, qj, pending, take_filler):
                # heads 2p (rows 0:64) and 2p+1 (rows 64:128) of group fs=p
                n_ki = 4 * qj + 4
                po0 = psO.tile([HD + 1, NT], f32, tag="o",
                               name=f"po{p}_{qj}_0")
                po1 = psO.tile([HD + 1, NT], f32, tag="o",
                               name=f"po{p}_{qj}_1")
                pts = {}

                def s_pair(ki):
                    col0 = 0 if ki < 4 * qj else (ki - 4 * qj) * P
                    N = NT - col0
                    ps = psS.tile([P, 2, NT], f32, tag="s2",
                                  name=f"pss{p}_{qj}_{ki}")
                    qs = slice(qj * NT + col0, (qj + 1) * NT)
                    ks = slice(ki * P, (ki + 1) * P)
                    nc.tensor.matmul(ps[:, 0, 0:N], kt[0:HD, p, ks],
                                     qt[0:HD, p, qs], start=True, stop=True)
                    nc.tensor.matmul(ps[:, 1, 0:N], kt[HD:P, p, ks],
                                     qt[HD:P, p, qs], start=True, stop=True)
                    pt = tp.tile([P, 2, NT], bf16, tag="pt", bufs=LAG + 1,
                                 name=f"pt{p}_{qj}_{ki}")
                    # one exp over both heads' banks (h0 cols N:512 are
                    # unused garbage, exp'd harmlessly)
                    w = NT + N
                    nc.scalar.activation(
                        pt[:].rearrange("p a n -> p (a n)")[:, 0:w],
                        ps[:].rearrange("p a n -> p (a n)")[:, 0:w],
                        mybir.ActivationFunctionType.Exp, scale=0.125)
                    if ki >= 4 * qj:
                        nc.gpsimd.tensor_mul(pt[:, 0, 0:P], pt[:, 0, 0:P],
                                             tri[:])
                        nc.gpsimd.tensor_mul(pt[:, 1, 0:P], pt[:, 1, 0:P],
                                             tri[:])
                    pts[ki] = (pt, col0, N)

                def o_pair(ki):
                    pt, col0, N = pts.pop(ki)
                    st = (ki == 0)
                    sp = (ki == n_ki - 1)
                    nc.tensor.matmul(po0[:, col0:NT], vaug[:, ki, 2 * p, :],
                                     pt[:, 0, 0:N], start=st, stop=sp)
                    nc.tensor.matmul(po1[:, col0:NT], vaug[:, ki, 2 * p + 1, :],
                                     pt[:, 1, 0:N], start=st, stop=sp)

                # the sibling pair's division runs deep into this pair so
                # its PE broadcasts never wait on the (slow, lane-starved)
                # reciprocals
                div_step = min(LAG + 6, n_ki + LAG - 1)
                for step in range(n_ki + LAG):
                    if step < n_ki:
                        s_pair(step)
                    if step == div_step and pending is not None:
                        pending()
                        pending = None
                    if step >= LAG:
                        o_pair(step - LAG)
                    if step >= 2:
                        # filler from step 2 on: steps 0-1 prime the exp
                        # pipe; 2..LAG-1 would otherwise stall on the S
                        # PSUM ring while the exp stream catches up
                        take_filler()

                # reciprocals issued now (DVE), division deferred so the PE
                # broadcast matmuls queue behind later attention work
                # so copies FIRST: they are the last po readers, and the
                # next pair's P@V reuses these PSUM banks (WAR) — putting
                # them behind a slow reciprocal stalls the PE there.
                sos = []
                for h, po in ((0, po0), (1, po1)):
                    so = tp.tile([HD, NT], f32, tag="so", bufs=4,
                                 name=f"so{p}_{qj}_{h}")
                    nc.vector.tensor_copy(so[:], po[0:HD, :])
                    sos.append(so)
                # Reciprocal as exp(-ln d) on Scalar: [1, N] ops are
                # lane-starved everywhere, but Scalar's Ln/Exp pair
                # (~1.2us) beats DVE's reciprocal (~3.3us), and keeping
                # them off DVE keeps its queue shallow for the so copies
                # that release the po banks (next pair's P@V WARs).
                rrs = []
                for h, po in ((0, po0), (1, po1)):
                    rrt = tp.tile([1, NT], bf16, tag="rr", bufs=4,
                                  name=f"rr{p}_{qj}_{h}")
                    lnt = tp.tile([1, NT], f32, tag="ln", bufs=4,
                                  name=f"ln{p}_{qj}_{h}")
                    nc.scalar.activation(lnt[:], po[HD:HD + 1, :],
                                         mybir.ActivationFunctionType.Ln)
                    nc.scalar.activation(rrt[:], lnt[:],
                                         mybir.ActivationFunctionType.Exp,
                                         scale=-1.0)
                    rrs.append((rrt, sos[h]))

                def division():
                    for h, (rrt, so) in enumerate(rrs):
                        pb = psW.tile([HD, NT], f32, tag="w",
                                      name=f"pb{p}_{qj}_{h}")
                        nc.tensor.matmul(pb[:], ones_r[:], rrt[:],
                                         start=True, stop=True)
                        nc.vector.tensor_mul(
                            ot[HD * h:HD * (h + 1), p,
                               qj * NT:(qj + 1) * NT],
                            so[:], pb[:])
                return division

            # ---- schedule ----
            for u in phase1_units(0):
                u()
            backlog = []
            for qj in range(QJ):
                # phase1(qj+1) must complete within this qj (its attention
                # needs it), but phase4 units are movable: hold them for the
                # last, largest qj, whose exp stream otherwise saturates
                # Scalar while the PE runs filler-dry (and HAM re-throttles
                # the PE clock below ~90% activity).
                filler = list(phase1_units(qj + 1)) if qj + 1 < QJ else []
                if qj == QJ - 1:
                    filler += backlog
                    backlog = []
                # on the last qj, hold back a few units: they cover the PE
                # while the final division chain (lane-starved scalar recip
                # -> PE broadcast -> DVE mul) drains before the tail phase4
                spare = 3 if qj == QJ - 1 else 2
                reserved = filler[len(filler) - spare:]
                filler = filler[:len(filler) - spare]
                n_points = 2 * (4 * qj + 4 + LAG - 2)
                state = [0, 0]  # points passed, units taken

                def take_filler(filler=filler, state=state,
                                n_points=n_points):
                    state[0] += 1
                    want = (len(filler) * state[0]) // n_points
                    while state[1] < want:
                        filler[state[1]]()
                        state[1] += 1

                div0 = attn_pair(0, qj, None, take_filler)
                div1 = attn_pair(1, qj, div0, take_filler)
                if qj == QJ - 1:
                    tailpz[0] = True
                    resv_dve[0] = True
                while state[1] < len(filler):
                    filler[state[1]]()
                    state[1] += 1
                for u in reserved:
                    u()
                div1()
                resv_dve[0] = False
                backlog += phase4_units(qj, tail=(qj == QJ - 1))
            for u in backlog:
                u()

    _legalize_single_wait(nc)
    return nc


def _make_in_maps(x, Wq, Wk, Wv, Wo):
    bf = ml_dtypes.bfloat16
    in_maps = []
    for c in range(N_CORES):
        b, g = divmod(c, 4)
        sl = slice(g * F, (g + 1) * F)
        in_maps.append({
            "xT": np.ascontiguousarray(np.asarray(x)[b].T).astype(bf),
            "WqT": np.ascontiguousarray(np.asarray(Wq)[sl, :].T).astype(bf),
            "WkT": np.ascontiguousarray(np.asarray(Wk)[sl, :].T).astype(bf),
            "WvT": np.ascontiguousarray(np.asarray(Wv)[sl, :].T).astype(bf),
            "WoT": np.ascontiguousarray(np.asarray(Wo)[:, sl].T).astype(bf),
        })
    return in_maps


def run(x, Wq, Wk, Wv, Wo, trace=False, trace_cores=None):
    nc = build_nc()
    in_maps = _make_in_maps(x, Wq, Wk, Wv, Wo)
    res = run_bass_kernel_spmd(nc, in_maps, list(range(N_CORES)), trace=trace,
                               trace_cores=trace_cores)
    out = np.zeros((B, T, D), np.float32)
    for c in range(N_CORES):
        out[c // 4] += res.results[c]["Z"].astype(np.float32)
    return out, res


def kernel(x, Wq, Wk, Wv, Wo):
    try:
        out, _ = run(x, Wq, Wk, Wv, Wo)
    except Exception:
        # one retry for transient device errors (e.g. a wedged core from a
        # prior run)
        out, _ = run(x, Wq, Wk, Wv, Wo)
    return out


# revision 71
# speedup vs baseline: 1.0043x; 1.0043x over previous
"""Multi-head causal attention (B=2, T=2048, D=1024, H=16) on 8 TRN2
NeuronCores: data parallel over batch x tensor parallel over head groups
(4 heads per core). Each core computes its group's Q/K/V projections,
causal attention, and a partial output projection; the host sums the 4
partials per batch element.

v2: bf16 operands end to end; paired-head S matmuls via PE row tiling
(two K=64 matmuls run concurrently in row groups 0/1); one fused exp per
head pair; causal masking on GpSimd; softmax reciprocals as exp(-ln d)
on Scalar; divisions deferred so their PE broadcasts never wait; all
output-projection work held as PE filler for the last (largest) q-block
where the exp stream saturates Scalar; coalesced input DMAs and PE
warm-up matmuls to bridge the NEFF preamble + first transfers; deep
Z-staging ring so output casts never wait on DMA completions, with
drain-tail PSUM tiles alternated into the freed attention banks.

Self-contained: builds the Bass/Tile kernel, runs it via
run_bass_kernel_spmd on cores 0-7, gathers on host.
"""
import numpy as np
import ml_dtypes

import concourse.bass as bass
import concourse.mybir as mybir
import concourse.tile as tile
from concourse.bass_utils import run_bass_kernel_spmd
from concourse.masks import make_identity, make_upper_triangular

P = 128
B, T, D = 2, 2048, 1024
H_LOCAL = 4          # heads per core
HD = 64              # head dim
F = H_LOCAL * HD     # 256 features per group
KO = D // P          # 8 contraction subtiles
NT = 512             # matmul moving width / PSUM bank
QJ = T // NT         # 4 q column tiles
KT = T // P          # 16 k row tiles
N_CORES = 8
LAG = 4              # S-matmul lookahead over P@V accumulation
N_WARM = 12          # PE warm-up matmuls (HAM un-throttle) during DMA wait

f32 = mybir.dt.float32
f32r = mybir.dt.float32r
bf16 = mybir.dt.bfloat16

_uid = [0]


def _legalize_single_wait(nc):
    # This walrus build accepts only ONE sem wait per instruction; hoist
    # extra waits onto single-wait NoOps placed just before the instruction.
    for fn in nc.m.functions:
        for bb in fn.blocks:
            new_list = []
            changed = False
            for inst in bb.instructions:
                si = inst.sync_info
                if si is not None and len(si.on_wait) > 1:
                    waits = list(si.on_wait)
                    for w in waits[:-1]:
                        _uid[0] += 1
                        new_list.append(mybir.InstNoOp(
                            name=f"I-waitsplit-{_uid[0]}",
                            engine=inst.engine,
                            sync_info=mybir.SyncInfo(on_wait=[w], on_update=[]),
                        ))
                    inst.sync_info = mybir.SyncInfo(
                        on_wait=[waits[-1]], on_update=list(si.on_update))
                    changed = True
                new_list.append(inst)
            if changed:
                bb.instructions.clear()
                bb.instructions.extend(new_list)


def build_nc():
    nc = bass.Bass(trn_type="TRN2", target_bir_lowering=False, debug=False,
                   num_devices=N_CORES)
    xT = nc.dram_tensor("xT", [D, T], bf16, kind="ExternalInput").ap()
    WqT = nc.dram_tensor("WqT", [D, F], bf16, kind="ExternalInput").ap()
    WkT = nc.dram_tensor("WkT", [D, F], bf16, kind="ExternalInput").ap()
    WvT = nc.dram_tensor("WvT", [D, F], bf16, kind="ExternalInput").ap()
    WoT = nc.dram_tensor("WoT", [F, D], bf16, kind="ExternalInput").ap()
    Z = nc.dram_tensor("Z", [T, D], bf16, kind="ExternalOutput").ap()

    xTr = xT.rearrange("(ko p) t -> p ko t", p=P)
    w_r = {
        "q": WqT.rearrange("(ko p) f -> p ko f", p=P),
        "k": WkT.rearrange("(ko p) f -> p ko f", p=P),
        "v": WvT.rearrange("(ko p) f -> p ko f", p=P),
    }

    with tile.TileContext(nc) as tc:
        with (
            tc.tile_pool(name="cw", bufs=1) as cw,
            tc.tile_pool(name="sb1", bufs=1) as sb1,
            tc.tile_pool(name="tp", bufs=4) as tp,
            tc.tile_pool(name="psS", bufs=2, space="PSUM") as psS,
            tc.tile_pool(name="psW", bufs=2, space="PSUM") as psW,
            tc.tile_pool(name="psO", bufs=2, space="PSUM") as psO,
        ):
            # ---- PE warm-up: matmuls on a zeroed tile while DMAs land ----
            zero512 = cw.tile([P, NT], bf16, tag="zero", name="zero512")
            nc.vector.memset(zero512[:], 0.0)
            for r in range(N_WARM // 4):
                for b in range(2):
                    wps = psS.tile([P, 2, NT], f32, tag="s2",
                                   name=f"warm{r}_{b}")
                    for half in range(2):
                        nc.tensor.matmul(wps[:, half], zero512[:, 0:P],
                                         zero512[:], start=True, stop=True)

            # ---- persistent constants / staging ----
            w_sb = {}
            for name in ("q", "k", "v"):
                w_sb[name] = sb1.tile([P, KO, F], bf16, tag=f"w{name}",
                                      name=f"w{name}")
            xt = sb1.tile([P, KO, T], bf16, tag="xt", name="xt")
            # issue order: earliest-needed first (wq+xt@qj0 gate the 1st
            # matmul).  Few LARGE transfers: each dma_start trigger costs
            # ~640ns on the issuing engine, so per-(ko) DMAs serialize the
            # whole input stream behind ~40 triggers.
            nc.sync.dma_start(w_sb["q"][:, 0:2], w_r["q"][:, 0:2])
            nc.sync.dma_start(xt[:, 0:2, 0:NT], xTr[:, 0:2, 0:NT])
            nc.sync.dma_start(w_sb["q"][:, 2:4], w_r["q"][:, 2:4])
            nc.sync.dma_start(xt[:, 2:4, 0:NT], xTr[:, 2:4, 0:NT])
            nc.sync.dma_start(w_sb["q"][:, 4:8], w_r["q"][:, 4:8])
            nc.sync.dma_start(xt[:, 4:8, 0:NT], xTr[:, 4:8, 0:NT])
            nc.sync.dma_start(w_sb["k"][:, 0:4], w_r["k"][:, 0:4])
            nc.sync.dma_start(w_sb["k"][:, 4:8], w_r["k"][:, 4:8])
            nc.sync.dma_start(w_sb["v"][:, 0:4], w_r["v"][:, 0:4])
            nc.sync.dma_start(w_sb["v"][:, 4:8], w_r["v"][:, 4:8])
            for qj in range(1, QJ):
                nc.sync.dma_start(xt[:, :, qj * NT:(qj + 1) * NT],
                                  xTr[:, :, qj * NT:(qj + 1) * NT])

            wo = cw.tile([P, F // P, D], bf16, tag="wo", name="wo")
            nc.gpsimd.dma_start(wo[:], WoT.rearrange("(fo p) d -> p fo d", p=P))
            # allowed[k_row, q_col] = q >= k (upper-triangular incl. diagonal)
            tri = cw.tile([P, P], bf16, tag="tri", name="tri")
            make_upper_triangular(nc, tri[:], val=1.0, diag=True)
            ident = cw.tile([P, P], bf16, tag="ident", name="ident")
            make_identity(nc, ident[:])
            ones_r = cw.tile([1, HD], bf16, tag="ones", name="ones")
            nc.gpsimd.memset(ones_r[:], 1.0)

            # Q/K packed 2 heads per 128 rows: rows 0:64 head 2fs, 64:128
            # head 2fs+1.  The S matmuls contract K=64 per head; the pair
            # runs concurrently in PE row groups (tile_position (0,0)/(64,0)
            # auto-derived from base partitions).
            qt = cw.tile([P, F // P, T], bf16, tag="qt", name="qt")
            kt = cw.tile([P, F // P, T], bf16, tag="kt", name="kt")

            # V with a ones column per head: [k-token, kt, head, 0:64]=V^T,
            # [..., 64]=1 (gives softmax denominators for free in P@V)
            vaug = cw.tile([P, KT, H_LOCAL, HD + 1], bf16, tag="vaug",
                           name="vaug")
            nc.gpsimd.memset(vaug[:, :, :, HD:HD + 1], 1.0)

            ot = cw.tile([P, F // P, T], bf16, tag="ot", name="ot")

            def phase1_units(qj):
                # filler units (~8 PE matmuls each) spliced into attention
                # so the PE stream never drains
                sl = slice(qj * NT, (qj + 1) * NT)
                vt = tp.tile([P, F // P, NT], bf16, tag="vt", bufs=2,
                             name=f"vt{qj}")
                units = []

                def proj(name, fs):
                    def emit():
                        ps = psW.tile([P, NT], f32, tag="w",
                                      name=f"ps_{name}{fs}_{qj}")
                        for ko in range(KO):
                            nc.tensor.matmul(
                                ps[:], w_sb[name][:, ko, fs * P:(fs + 1) * P],
                                xt[:, ko, sl],
                                start=(ko == 0), stop=(ko == KO - 1))
                        if name == "q":
                            nc.vector.tensor_copy(qt[:, fs, sl], ps[:])
                        elif name == "k":
                            nc.vector.tensor_copy(kt[:, fs, sl], ps[:])
                        else:
                            nc.vector.tensor_copy(vt[:, fs, :], ps[:])
                    return emit

                def vtrans(fs):
                    def emit():
                        pst = psW.tile([P, 4, P], bf16, tag="w",
                                       name=f"pvt{fs}_{qj}")
                        for j in range(4):
                            nc.tensor.transpose(
                                pst[:, j], vt[:, fs, j * P:(j + 1) * P],
                                ident[:])
                        nc.vector.tensor_copy(
                            vaug[:, 4 * qj:4 * qj + 4, 2 * fs:2 * fs + 2,
                                 0:HD],
                            pst[:].rearrange("p j (a b) -> p j a b", a=2))
                    return emit

                for name in ("q", "k", "v"):
                    for fs in range(F // P):
                        units.append(proj(name, fs))
                for fs in range(F // P):
                    units.append(vtrans(fs))
                return units

            # after the last attention pair the 4 psS banks are free:
            # alternating tail pz tiles between psW and psS doubles the
            # effective ring depth, so the drain-tail phase4 chain is not
            # serialized on each cast two slots back
            tailpz = [False, 0]
            # while the final division's lane-starved Ln/Exp reciprocals
            # drain on Scalar, filler casts must not queue ahead of them
            resv_dve = [False]

            def phase4_units(qj, tail=False):
                units = []

                def unit(qt_i, dt):
                    def emit():
                        tailpz[1] += 1
                        if tailpz[0] and tailpz[1] % 2 == 0:
                            pz = psS.tile([P, NT], f32, tag="s2",
                                          name=f"pz{qt_i}_{dt}")
                        else:
                            pz = psW.tile([P, NT], f32, tag="w",
                                          name=f"pz{qt_i}_{dt}")
                        for fs in range(F // P):
                            nc.tensor.matmul(
                                pz[:], ot[:, fs, qt_i * P:(qt_i + 1) * P],
                                wo[:, fs, dt * NT:(dt + 1) * NT],
                                start=(fs == 0), stop=(fs == F // P - 1))
                        zs = tp.tile([P, NT], bf16, tag="z", bufs=8,
                                     name=f"zs{qt_i}_{dt}")
                        if dt == 1 and not resv_dve[0]:
                            # alternate the PSUM->SBUF casts across both
                            # engines: halves the psW WAR chain latency;
                            # Scalar has enough bubbles even while the exp
                            # stream runs, and is idle in the drain tail
                            nc.scalar.activation(
                                zs[:], pz[:],
                                mybir.ActivationFunctionType.Copy)
                        else:
                            nc.vector.tensor_copy(zs[:], pz[:])
                        # alternate trigger queues so the output stream is
                        # not serialized behind one engine's ~640ns triggers
                        # (in the tail, keep everything on sync so gpsimd's
                        # software DMA queue can drain early)
                        if tail:
                            eng = nc.sync if (qt_i + dt) % 2 == 0 \
                                else nc.scalar
                        else:
                            eng = nc.sync if (qt_i + dt) % 2 == 0 \
                                else nc.gpsimd
                        eng.dma_start(
                            Z[qt_i * P:(qt_i + 1) * P,
                              dt * NT:(dt + 1) * NT], zs[:])
                    return emit

                for qt_i in range(4 * qj, 4 * qj + 4):
                    for dt in range(D // NT):
                        units.append(unit(qt_i, dt))
                return units

            def attn_pair(p, qj, pending, take_filler):
                # heads 2p (rows 0:64) and 2p+1 (rows 64:128) of group fs=p
                n_ki = 4 * qj + 4
                po0 = psO.tile([HD + 1, NT], f32, tag="o",
                               name=f"po{p}_{qj}_0")
                po1 = psO.tile([HD + 1, NT], f32, tag="o",
                               name=f"po{p}_{qj}_1")
                pts = {}

                def s_pair(ki):
                    col0 = 0 if ki < 4 * qj else (ki - 4 * qj) * P
                    N = NT - col0
                    ps = psS.tile([P, 2, NT], f32, tag="s2",
                                  name=f"pss{p}_{qj}_{ki}")
                    qs = slice(qj * NT + col0, (qj + 1) * NT)
                    ks = slice(ki * P, (ki + 1) * P)
                    nc.tensor.matmul(ps[:, 0, 0:N], kt[0:HD, p, ks],
                                     qt[0:HD, p, qs], start=True, stop=True)
                    nc.tensor.matmul(ps[:, 1, 0:N], kt[HD:P, p, ks],
                                     qt[HD:P, p, qs], start=True, stop=True)
                    pt = tp.tile([P, 2, NT], bf16, tag="pt", bufs=LAG + 1,
                                 name=f"pt{p}_{qj}_{ki}")
                    # one exp over both heads' banks (h0 cols N:512 are
                    # unused garbage, exp'd harmlessly)
                    w = NT + N
                    nc.scalar.activation(
                        pt[:].rearrange("p a n -> p (a n)")[:, 0:w],
                        ps[:].rearrange("p a n -> p (a n)")[:, 0:w],
                        mybir.ActivationFunctionType.Exp, scale=0.125)
                    if ki >= 4 * qj:
                        nc.gpsimd.tensor_mul(pt[:, 0, 0:P], pt[:, 0, 0:P],
                                             tri[:])
                        nc.gpsimd.tensor_mul(pt[:, 1, 0:P], pt[:, 1, 0:P],
                                             tri[:])
                    pts[ki] = (pt, col0, N)

                def o_pair(ki):
                    pt, col0, N = pts.pop(ki)
                    st = (ki == 0)
                    sp = (ki == n_ki - 1)
                    nc.tensor.matmul(po0[:, col0:NT], vaug[:, ki, 2 * p, :],
                                     pt[:, 0, 0:N], start=st, stop=sp)
                    nc.tensor.matmul(po1[:, col0:NT], vaug[:, ki, 2 * p + 1, :],
                                     pt[:, 1, 0:N], start=st, stop=sp)

                # the sibling pair's division runs deep into this pair so
                # its PE broadcasts never wait on the (slow, lane-starved)
                # reciprocals
                div_step = min(LAG + 5, n_ki + LAG - 1)
                for step in range(n_ki + LAG):
                    if step < n_ki:
                        s_pair(step)
                    if step == div_step and pending is not None:
                        pending()
                        pending = None
                    if step >= LAG:
                        o_pair(step - LAG)
                    if step >= 2:
                        # filler from step 2 on: steps 0-1 prime the exp
                        # pipe; 2..LAG-1 would otherwise stall on the S
                        # PSUM ring while the exp stream catches up
                        take_filler()

                # reciprocals issued now (DVE), division deferred so the PE
                # broadcast matmuls queue behind later attention work
                # so copies FIRST: they are the last po readers, and the
                # next pair's P@V reuses these PSUM banks (WAR) — putting
                # them behind a slow reciprocal stalls the PE there.
                sos = []
                for h, po in ((0, po0), (1, po1)):
                    so = tp.tile([HD, NT], f32, tag="so", bufs=4,
                                 name=f"so{p}_{qj}_{h}")
                    nc.vector.tensor_copy(so[:], po[0:HD, :])
                    sos.append(so)
                # Reciprocal as exp(-ln d) on Scalar: [1, N] ops are
                # lane-starved everywhere, but Scalar's Ln/Exp pair
                # (~1.2us) beats DVE's reciprocal (~3.3us), and keeping
                # them off DVE keeps its queue shallow for the so copies
                # that release the po banks (next pair's P@V WARs).
                rrs = []
                for h, po in ((0, po0), (1, po1)):
                    rrt = tp.tile([1, NT], bf16, tag="rr", bufs=4,
                                  name=f"rr{p}_{qj}_{h}")
                    lnt = tp.tile([1, NT], f32, tag="ln", bufs=4,
                                  name=f"ln{p}_{qj}_{h}")
                    nc.scalar.activation(lnt[:], po[HD:HD + 1, :],
                                         mybir.ActivationFunctionType.Ln)
                    nc.scalar.activation(rrt[:], lnt[:],
                                         mybir.ActivationFunctionType.Exp,
                                         scale=-1.0)
                    rrs.append((rrt, sos[h]))

                def division():
                    for h, (rrt, so) in enumerate(rrs):
                        pb = psW.tile([HD, NT], f32, tag="w",
                                      name=f"pb{p}_{qj}_{h}")
                        nc.tensor.matmul(pb[:], ones_r[:], rrt[:],
                                         start=True, stop=True)
                        nc.vector.tensor_mul(
                            ot[HD * h:HD * (h + 1), p,
                               qj * NT:(qj + 1) * NT],
                            so[:], pb[:])
                return division

            # ---- schedule ----
            for u in phase1_units(0):
                u()
            backlog = []
            for qj in range(QJ):
                # phase1(qj+1) must complete within this qj (its attention
                # needs it), but phase4 units are movable: hold them for the
                # last, largest qj, whose exp stream otherwise saturates
                # Scalar while the PE runs filler-dry (and HAM re-throttles
                # the PE clock below ~90% activity).
                filler = list(phase1_units(qj + 1)) if qj + 1 < QJ else []
                if qj == QJ - 1:
                    filler += backlog
                    backlog = []
                # on the last qj, hold back a few units: they cover the PE
                # while the final division chain (lane-starved scalar recip
                # -> PE broadcast -> DVE mul) drains before the tail phase4
                spare = 4 if qj == QJ - 1 else 2
                reserved = filler[len(filler) - spare:]
                filler = filler[:len(filler) - spare]
                n_points = 2 * (4 * qj + 4 + LAG - 2)
                state = [0, 0]  # points passed, units taken

                def take_filler(filler=filler, state=state,
                                n_points=n_points):
                    state[0] += 1
                    want = (len(filler) * state[0]) // n_points
                    while state[1] < want:
                        filler[state[1]]()
                        state[1] += 1

                div0 = attn_pair(0, qj, None, take_filler)
                div1 = attn_pair(1, qj, div0, take_filler)
                if qj == QJ - 1:
                    tailpz[0] = True
                    resv_dve[0] = True
                while state[1] < len(filler):
                    filler[state[1]]()
                    state[1] += 1
                for u in reserved:
                    u()
                div1()
                resv_dve[0] = False
                backlog += phase4_units(qj, tail=(qj == QJ - 1))
            for u in backlog:
                u()

    _legalize_single_wait(nc)
    return nc


def _make_in_maps(x, Wq, Wk, Wv, Wo):
    bf = ml_dtypes.bfloat16
    in_maps = []
    for c in range(N_CORES):
        b, g = divmod(c, 4)
        sl = slice(g * F, (g + 1) * F)
        in_maps.append({
            "xT": np.ascontiguousarray(np.asarray(x)[b].T).astype(bf),
            "WqT": np.ascontiguousarray(np.asarray(Wq)[sl, :].T).astype(bf),
            "WkT": np.ascontiguousarray(np.asarray(Wk)[sl, :].T).astype(bf),
            "WvT": np.ascontiguousarray(np.asarray(Wv)[sl, :].T).astype(bf),
            "WoT": np.ascontiguousarray(np.asarray(Wo)[:, sl].T).astype(bf),
        })
    return in_maps


def run(x, Wq, Wk, Wv, Wo, trace=False, trace_cores=None):
    nc = build_nc()
    in_maps = _make_in_maps(x, Wq, Wk, Wv, Wo)
    res = run_bass_kernel_spmd(nc, in_maps, list(range(N_CORES)), trace=trace,
                               trace_cores=trace_cores)
    out = np.zeros((B, T, D), np.float32)
    for c in range(N_CORES):
        out[c // 4] += res.results[c]["Z"].astype(np.float32)
    return out, res


def kernel(x, Wq, Wk, Wv, Wo):
    try:
        out, _ = run(x, Wq, Wk, Wv, Wo)
    except Exception:
        # one retry for transient device errors (e.g. a wedged core from a
        # prior run)
        out, _ = run(x, Wq, Wk, Wv, Wo)
    return out


# revision 72
# speedup vs baseline: 1.0194x; 1.0151x over previous
"""Multi-head causal attention (B=2, T=2048, D=1024, H=16) on 8 TRN2
NeuronCores: data parallel over batch x tensor parallel over head groups
(4 heads per core). Each core computes its group's Q/K/V projections,
causal attention, and a partial output projection; the host sums the 4
partials per batch element.

v2: bf16 operands end to end; paired-head S matmuls via PE row tiling
(two K=64 matmuls run concurrently in row groups 0/1); one fused exp per
head pair; causal masking on GpSimd; softmax reciprocals as exp(-ln d)
on Scalar; divisions deferred so their PE broadcasts never wait; all
output-projection work held as PE filler for the last (largest) q-block
where the exp stream saturates Scalar; coalesced input DMAs and PE
warm-up matmuls to bridge the NEFF preamble + first transfers; deep
Z-staging ring so output casts never wait on DMA completions, with
drain-tail PSUM tiles alternated into the freed attention banks.

Self-contained: builds the Bass/Tile kernel, runs it via
run_bass_kernel_spmd on cores 0-7, gathers on host.
"""
import numpy as np
import ml_dtypes

import concourse.bass as bass
import concourse.mybir as mybir
import concourse.tile as tile
from concourse.bass_utils import run_bass_kernel_spmd
from concourse.masks import make_identity, make_upper_triangular

P = 128
B, T, D = 2, 2048, 1024
H_LOCAL = 4          # heads per core
HD = 64              # head dim
F = H_LOCAL * HD     # 256 features per group
KO = D // P          # 8 contraction subtiles
NT = 512             # matmul moving width / PSUM bank
QJ = T // NT         # 4 q column tiles
KT = T // P          # 16 k row tiles
N_CORES = 8
LAG = 4              # S-matmul lookahead over P@V accumulation
N_WARM = 12          # PE warm-up matmuls (HAM un-throttle) during DMA wait

f32 = mybir.dt.float32
f32r = mybir.dt.float32r
bf16 = mybir.dt.bfloat16

_uid = [0]


def _legalize_single_wait(nc):
    # This walrus build accepts only ONE sem wait per instruction; hoist
    # extra waits onto single-wait NoOps placed just before the instruction.
    for fn in nc.m.functions:
        for bb in fn.blocks:
            new_list = []
            changed = False
            for inst in bb.instructions:
                si = inst.sync_info
                if si is not None and len(si.on_wait) > 1:
                    waits = list(si.on_wait)
                    for w in waits[:-1]:
                        _uid[0] += 1
                        new_list.append(mybir.InstNoOp(
                            name=f"I-waitsplit-{_uid[0]}",
                            engine=inst.engine,
                            sync_info=mybir.SyncInfo(on_wait=[w], on_update=[]),
                        ))
                    inst.sync_info = mybir.SyncInfo(
                        on_wait=[waits[-1]], on_update=list(si.on_update))
                    changed = True
                new_list.append(inst)
            if changed:
                bb.instructions.clear()
                bb.instructions.extend(new_list)


def build_nc():
    nc = bass.Bass(trn_type="TRN2", target_bir_lowering=False, debug=False,
                   num_devices=N_CORES)
    xT = nc.dram_tensor("xT", [D, T], bf16, kind="ExternalInput").ap()
    WqT = nc.dram_tensor("WqT", [D, F], bf16, kind="ExternalInput").ap()
    WkT = nc.dram_tensor("WkT", [D, F], bf16, kind="ExternalInput").ap()
    WvT = nc.dram_tensor("WvT", [D, F], bf16, kind="ExternalInput").ap()
    WoT = nc.dram_tensor("WoT", [F, D], bf16, kind="ExternalInput").ap()
    Z = nc.dram_tensor("Z", [T, D], bf16, kind="ExternalOutput").ap()

    xTr = xT.rearrange("(ko p) t -> p ko t", p=P)
    w_r = {
        "q": WqT.rearrange("(ko p) f -> p ko f", p=P),
        "k": WkT.rearrange("(ko p) f -> p ko f", p=P),
        "v": WvT.rearrange("(ko p) f -> p ko f", p=P),
    }

    with tile.TileContext(nc) as tc:
        with (
            tc.tile_pool(name="cw", bufs=1) as cw,
            tc.tile_pool(name="sb1", bufs=1) as sb1,
            tc.tile_pool(name="tp", bufs=4) as tp,
            tc.tile_pool(name="psS", bufs=2, space="PSUM") as psS,
            tc.tile_pool(name="psW", bufs=2, space="PSUM") as psW,
            tc.tile_pool(name="psO", bufs=2, space="PSUM") as psO,
        ):
            # ---- PE warm-up: matmuls on a zeroed tile while DMAs land ----
            zero512 = cw.tile([P, NT], bf16, tag="zero", name="zero512")
            nc.vector.memset(zero512[:], 0.0)
            for r in range(N_WARM // 4):
                for b in range(2):
                    wps = psS.tile([P, 2, NT], f32, tag="s2",
                                   name=f"warm{r}_{b}")
                    for half in range(2):
                        nc.tensor.matmul(wps[:, half], zero512[:, 0:P],
                                         zero512[:], start=True, stop=True)

            # ---- persistent constants / staging ----
            w_sb = {}
            for name in ("q", "k", "v"):
                w_sb[name] = sb1.tile([P, KO, F], bf16, tag=f"w{name}",
                                      name=f"w{name}")
            xt = sb1.tile([P, KO, T], bf16, tag="xt", name="xt")
            # issue order: earliest-needed first (wq+xt@qj0 gate the 1st
            # matmul).  Few LARGE transfers: each dma_start trigger costs
            # ~640ns on the issuing engine, so per-(ko) DMAs serialize the
            # whole input stream behind ~40 triggers.
            nc.sync.dma_start(w_sb["q"][:, 0:2], w_r["q"][:, 0:2])
            nc.sync.dma_start(xt[:, 0:2, 0:NT], xTr[:, 0:2, 0:NT])
            nc.sync.dma_start(w_sb["q"][:, 2:4], w_r["q"][:, 2:4])
            nc.sync.dma_start(xt[:, 2:4, 0:NT], xTr[:, 2:4, 0:NT])
            nc.sync.dma_start(w_sb["q"][:, 4:8], w_r["q"][:, 4:8])
            nc.sync.dma_start(xt[:, 4:8, 0:NT], xTr[:, 4:8, 0:NT])
            nc.sync.dma_start(w_sb["k"][:, 0:4], w_r["k"][:, 0:4])
            nc.sync.dma_start(w_sb["k"][:, 4:8], w_r["k"][:, 4:8])
            nc.sync.dma_start(w_sb["v"][:, 0:4], w_r["v"][:, 0:4])
            nc.sync.dma_start(w_sb["v"][:, 4:8], w_r["v"][:, 4:8])
            for qj in range(1, QJ):
                nc.sync.dma_start(xt[:, :, qj * NT:(qj + 1) * NT],
                                  xTr[:, :, qj * NT:(qj + 1) * NT])

            wo = cw.tile([P, F // P, D], bf16, tag="wo", name="wo")
            nc.gpsimd.dma_start(wo[:], WoT.rearrange("(fo p) d -> p fo d", p=P))
            # allowed[k_row, q_col] = q >= k (upper-triangular incl. diagonal)
            tri = cw.tile([P, P], bf16, tag="tri", name="tri")
            make_upper_triangular(nc, tri[:], val=1.0, diag=True)
            ident = cw.tile([P, P], bf16, tag="ident", name="ident")
            make_identity(nc, ident[:])
            ones_r = cw.tile([1, HD], bf16, tag="ones", name="ones")
            nc.gpsimd.memset(ones_r[:], 1.0)

            # Q/K packed 2 heads per 128 rows: rows 0:64 head 2fs, 64:128
            # head 2fs+1.  The S matmuls contract K=64 per head; the pair
            # runs concurrently in PE row groups (tile_position (0,0)/(64,0)
            # auto-derived from base partitions).
            qt = cw.tile([P, F // P, T], bf16, tag="qt", name="qt")
            kt = cw.tile([P, F // P, T], bf16, tag="kt", name="kt")

            # V with a ones column per head: [k-token, kt, head, 0:64]=V^T,
            # [..., 64]=1 (gives softmax denominators for free in P@V)
            vaug = cw.tile([P, KT, H_LOCAL, HD + 1], bf16, tag="vaug",
                           name="vaug")
            nc.gpsimd.memset(vaug[:, :, :, HD:HD + 1], 1.0)

            ot = cw.tile([P, F // P, T], bf16, tag="ot", name="ot")

            def phase1_units(qj):
                # filler units (~8 PE matmuls each) spliced into attention
                # so the PE stream never drains
                sl = slice(qj * NT, (qj + 1) * NT)
                vt = tp.tile([P, F // P, NT], bf16, tag="vt", bufs=2,
                             name=f"vt{qj}")
                units = []

                def proj(name, fs):
                    def emit():
                        ps = psW.tile([P, NT], f32, tag="w",
                                      name=f"ps_{name}{fs}_{qj}")
                        for ko in range(KO):
                            nc.tensor.matmul(
                                ps[:], w_sb[name][:, ko, fs * P:(fs + 1) * P],
                                xt[:, ko, sl],
                                start=(ko == 0), stop=(ko == KO - 1))
                        if name == "q":
                            nc.vector.tensor_copy(qt[:, fs, sl], ps[:])
                        elif name == "k":
                            nc.vector.tensor_copy(kt[:, fs, sl], ps[:])
                        else:
                            nc.vector.tensor_copy(vt[:, fs, :], ps[:])
                    return emit

                def vtrans(fs):
                    def emit():
                        pst = psW.tile([P, 4, P], bf16, tag="w",
                                       name=f"pvt{fs}_{qj}")
                        for j in range(4):
                            nc.tensor.transpose(
                                pst[:, j], vt[:, fs, j * P:(j + 1) * P],
                                ident[:])
                        nc.vector.tensor_copy(
                            vaug[:, 4 * qj:4 * qj + 4, 2 * fs:2 * fs + 2,
                                 0:HD],
                            pst[:].rearrange("p j (a b) -> p j a b", a=2))
                    return emit

                for name in ("q", "k", "v"):
                    for fs in range(F // P):
                        units.append(proj(name, fs))
                for fs in range(F // P):
                    units.append(vtrans(fs))
                return units

            # after the last attention pair the 4 psS banks are free:
            # alternating tail pz tiles between psW and psS doubles the
            # effective ring depth, so the drain-tail phase4 chain is not
            # serialized on each cast two slots back
            tailpz = [False, 0]
            # while the final division's lane-starved Ln/Exp reciprocals
            # drain on Scalar, filler casts must not queue ahead of them
            resv_dve = [False]

            def phase4_units(qj, tail=False):
                units = []

                def unit(qt_i, dt):
                    def emit():
                        tailpz[1] += 1
                        if tailpz[0] and tailpz[1] % 2 == 0:
                            pz = psS.tile([P, NT], f32, tag="s2",
                                          name=f"pz{qt_i}_{dt}")
                        else:
                            pz = psW.tile([P, NT], f32, tag="w",
                                          name=f"pz{qt_i}_{dt}")
                        for fs in range(F // P):
                            nc.tensor.matmul(
                                pz[:], ot[:, fs, qt_i * P:(qt_i + 1) * P],
                                wo[:, fs, dt * NT:(dt + 1) * NT],
                                start=(fs == 0), stop=(fs == F // P - 1))
                        zs = tp.tile([P, NT], bf16, tag="z", bufs=8,
                                     name=f"zs{qt_i}_{dt}")
                        if dt == 1 and not resv_dve[0]:
                            # alternate the PSUM->SBUF casts across both
                            # engines: halves the psW WAR chain latency;
                            # Scalar has enough bubbles even while the exp
                            # stream runs, and is idle in the drain tail
                            nc.scalar.activation(
                                zs[:], pz[:],
                                mybir.ActivationFunctionType.Copy)
                        else:
                            nc.vector.tensor_copy(zs[:], pz[:])
                        # alternate trigger queues so the output stream is
                        # not serialized behind one engine's ~640ns triggers
                        # (in the tail, keep everything on sync so gpsimd's
                        # software DMA queue can drain early)
                        if tail:
                            eng = nc.sync if (qt_i + dt) % 2 == 0 \
                                else nc.scalar
                        else:
                            eng = nc.sync if (qt_i + dt) % 2 == 0 \
                                else nc.gpsimd
                        eng.dma_start(
                            Z[qt_i * P:(qt_i + 1) * P,
                              dt * NT:(dt + 1) * NT], zs[:])
                    return emit

                for qt_i in range(4 * qj, 4 * qj + 4):
                    for dt in range(D // NT):
                        units.append(unit(qt_i, dt))
                return units

            def attn_pair(p, qj, pending, take_filler):
                # heads 2p (rows 0:64) and 2p+1 (rows 64:128) of group fs=p
                n_ki = 4 * qj + 4
                po0 = psO.tile([HD + 1, NT], f32, tag="o",
                               name=f"po{p}_{qj}_0")
                po1 = psO.tile([HD + 1, NT], f32, tag="o",
                               name=f"po{p}_{qj}_1")
                pts = {}

                def s_pair(ki):
                    col0 = 0 if ki < 4 * qj else (ki - 4 * qj) * P
                    N = NT - col0
                    ps = psS.tile([P, 2, NT], f32, tag="s2",
                                  name=f"pss{p}_{qj}_{ki}")
                    qs = slice(qj * NT + col0, (qj + 1) * NT)
                    ks = slice(ki * P, (ki + 1) * P)
                    nc.tensor.matmul(ps[:, 0, 0:N], kt[0:HD, p, ks],
                                     qt[0:HD, p, qs], start=True, stop=True)
                    nc.tensor.matmul(ps[:, 1, 0:N], kt[HD:P, p, ks],
                                     qt[HD:P, p, qs], start=True, stop=True)
                    pt = tp.tile([P, 2, NT], bf16, tag="pt", bufs=LAG + 1,
                                 name=f"pt{p}_{qj}_{ki}")
                    # one exp over both heads' banks (h0 cols N:512 are
                    # unused garbage, exp'd harmlessly)
                    w = NT + N
                    nc.scalar.activation(
                        pt[:].rearrange("p a n -> p (a n)")[:, 0:w],
                        ps[:].rearrange("p a n -> p (a n)")[:, 0:w],
                        mybir.ActivationFunctionType.Exp, scale=0.125)
                    if ki >= 4 * qj:
                        nc.gpsimd.tensor_mul(pt[:, 0, 0:P], pt[:, 0, 0:P],
                                             tri[:])
                        nc.gpsimd.tensor_mul(pt[:, 1, 0:P], pt[:, 1, 0:P],
                                             tri[:])
                    pts[ki] = (pt, col0, N)

                def o_pair(ki):
                    pt, col0, N = pts.pop(ki)
                    st = (ki == 0)
                    sp = (ki == n_ki - 1)
                    nc.tensor.matmul(po0[:, col0:NT], vaug[:, ki, 2 * p, :],
                                     pt[:, 0, 0:N], start=st, stop=sp)
                    nc.tensor.matmul(po1[:, col0:NT], vaug[:, ki, 2 * p + 1, :],
                                     pt[:, 1, 0:N], start=st, stop=sp)

                # the sibling pair's division runs deep into this pair so
                # its PE broadcasts never wait on the (slow, lane-starved)
                # reciprocals
                div_step = min(LAG + 6, n_ki + LAG - 1)
                for step in range(n_ki + LAG):
                    if step < n_ki:
                        s_pair(step)
                    if step == div_step and pending is not None:
                        pending()
                        pending = None
                    if step >= LAG:
                        o_pair(step - LAG)
                    if step >= 2:
                        # filler from step 2 on: steps 0-1 prime the exp
                        # pipe; 2..LAG-1 would otherwise stall on the S
                        # PSUM ring while the exp stream catches up
                        take_filler()

                # reciprocals issued now (DVE), division deferred so the PE
                # broadcast matmuls queue behind later attention work
                # so copies FIRST: they are the last po readers, and the
                # next pair's P@V reuses these PSUM banks (WAR) — putting
                # them behind a slow reciprocal stalls the PE there.
                sos = []
                for h, po in ((0, po0), (1, po1)):
                    so = tp.tile([HD, NT], f32, tag="so", bufs=4,
                                 name=f"so{p}_{qj}_{h}")
                    nc.vector.tensor_copy(so[:], po[0:HD, :])
                    sos.append(so)
                # Reciprocal as exp(-ln d) on Scalar: [1, N] ops are
                # lane-starved everywhere, but Scalar's Ln/Exp pair
                # (~1.2us) beats DVE's reciprocal (~3.3us), and keeping
                # them off DVE keeps its queue shallow for the so copies
                # that release the po banks (next pair's P@V WARs).
                rrs = []
                for h, po in ((0, po0), (1, po1)):
                    rrt = tp.tile([1, NT], bf16, tag="rr", bufs=4,
                                  name=f"rr{p}_{qj}_{h}")
                    lnt = tp.tile([1, NT], f32, tag="ln", bufs=4,
                                  name=f"ln{p}_{qj}_{h}")
                    nc.scalar.activation(lnt[:], po[HD:HD + 1, :],
                                         mybir.ActivationFunctionType.Ln)
                    nc.scalar.activation(rrt[:], lnt[:],
                                         mybir.ActivationFunctionType.Exp,
                                         scale=-1.0)
                    rrs.append((rrt, sos[h]))

                def division():
                    for h, (rrt, so) in enumerate(rrs):
                        pb = psW.tile([HD, NT], f32, tag="w",
                                      name=f"pb{p}_{qj}_{h}")
                        nc.tensor.matmul(pb[:], ones_r[:], rrt[:],
                                         start=True, stop=True)
                        nc.vector.tensor_mul(
                            ot[HD * h:HD * (h + 1), p,
                               qj * NT:(qj + 1) * NT],
                            so[:], pb[:])
                return division

            # ---- schedule ----
            for u in phase1_units(0):
                u()
            backlog = []
            for qj in range(QJ):
                # phase1(qj+1) must complete within this qj (its attention
                # needs it), but phase4 units are movable: hold them for the
                # last, largest qj, whose exp stream otherwise saturates
                # Scalar while the PE runs filler-dry (and HAM re-throttles
                # the PE clock below ~90% activity).
                filler = list(phase1_units(qj + 1)) if qj + 1 < QJ else []
                if qj == QJ - 1:
                    filler += backlog
                    backlog = []
                # on the last qj, hold back a few units: they cover the PE
                # while the final division chain (lane-starved scalar recip
                # -> PE broadcast -> DVE mul) drains before the tail phase4
                spare = 4 if qj == QJ - 1 else 2
                reserved = filler[len(filler) - spare:]
                filler = filler[:len(filler) - spare]
                n_points = 2 * (4 * qj + 4 + LAG - 2)
                state = [0, 0]  # points passed, units taken

                def take_filler(filler=filler, state=state,
                                n_points=n_points):
                    state[0] += 1
                    want = (len(filler) * state[0]) // n_points
                    while state[1] < want:
                        filler[state[1]]()
                        state[1] += 1

                div0 = attn_pair(0, qj, None, take_filler)
                div1 = attn_pair(1, qj, div0, take_filler)
                if qj == QJ - 1:
                    tailpz[0] = True
                    resv_dve[0] = True
                while state[1] < len(filler):
                    filler[state[1]]()
                    state[1] += 1
                for u in reserved:
                    u()
                div1()
                resv_dve[0] = False
                backlog += phase4_units(qj, tail=(qj == QJ - 1))
            for u in backlog:
                u()

    _legalize_single_wait(nc)
    return nc


def _make_in_maps(x, Wq, Wk, Wv, Wo):
    bf = ml_dtypes.bfloat16
    in_maps = []
    for c in range(N_CORES):
        b, g = divmod(c, 4)
        sl = slice(g * F, (g + 1) * F)
        in_maps.append({
            "xT": np.ascontiguousarray(np.asarray(x)[b].T).astype(bf),
            "WqT": np.ascontiguousarray(np.asarray(Wq)[sl, :].T).astype(bf),
            "WkT": np.ascontiguousarray(np.asarray(Wk)[sl, :].T).astype(bf),
            "WvT": np.ascontiguousarray(np.asarray(Wv)[sl, :].T).astype(bf),
            "WoT": np.ascontiguousarray(np.asarray(Wo)[:, sl].T).astype(bf),
        })
    return in_maps


def run(x, Wq, Wk, Wv, Wo, trace=False, trace_cores=None):
    nc = build_nc()
    in_maps = _make_in_maps(x, Wq, Wk, Wv, Wo)
    res = run_bass_kernel_spmd(nc, in_maps, list(range(N_CORES)), trace=trace,
                               trace_cores=trace_cores)
    out = np.zeros((B, T, D), np.float32)
    for c in range(N_CORES):
        out[c // 4] += res.results[c]["Z"].astype(np.float32)
    return out, res


def kernel(x, Wq, Wk, Wv, Wo):
    try:
        out, _ = run(x, Wq, Wk, Wv, Wo)
    except Exception:
        # one retry for transient device errors (e.g. a wedged core from a
        # prior run)
        out, _ = run(x, Wq, Wk, Wv, Wo)
    return out


# revision 73
# speedup vs baseline: 1.0253x; 1.0058x over previous
"""Multi-head causal attention (B=2, T=2048, D=1024, H=16) on 8 TRN2
NeuronCores: data parallel over batch x tensor parallel over head groups
(4 heads per core). Each core computes its group's Q/K/V projections,
causal attention, and a partial output projection; the host sums the 4
partials per batch element.

v2: bf16 operands end to end; paired-head S matmuls via PE row tiling
(two K=64 matmuls run concurrently in row groups 0/1); one fused exp per
head pair; causal masking on GpSimd; softmax reciprocals as exp(-ln d)
on Scalar; divisions deferred so their PE broadcasts never wait; all
output-projection work held as PE filler for the last (largest) q-block
where the exp stream saturates Scalar; coalesced input DMAs and PE
warm-up matmuls to bridge the NEFF preamble + first transfers; deep
Z-staging ring so output casts never wait on DMA completions, with
drain-tail PSUM tiles alternated into the freed attention banks.

Self-contained: builds the Bass/Tile kernel, runs it via
run_bass_kernel_spmd on cores 0-7, gathers on host.
"""
import numpy as np
import ml_dtypes

import concourse.bass as bass
import concourse.mybir as mybir
import concourse.tile as tile
from concourse.bass_utils import run_bass_kernel_spmd
from concourse.masks import make_identity, make_upper_triangular

P = 128
B, T, D = 2, 2048, 1024
H_LOCAL = 4          # heads per core
HD = 64              # head dim
F = H_LOCAL * HD     # 256 features per group
KO = D // P          # 8 contraction subtiles
NT = 512             # matmul moving width / PSUM bank
QJ = T // NT         # 4 q column tiles
KT = T // P          # 16 k row tiles
N_CORES = 8
LAG = 4              # S-matmul lookahead over P@V accumulation
N_WARM = 12          # PE warm-up matmuls (HAM un-throttle) during DMA wait

f32 = mybir.dt.float32
f32r = mybir.dt.float32r
bf16 = mybir.dt.bfloat16

_uid = [0]


def _legalize_single_wait(nc):
    # This walrus build accepts only ONE sem wait per instruction; hoist
    # extra waits onto single-wait NoOps placed just before the instruction.
    for fn in nc.m.functions:
        for bb in fn.blocks:
            new_list = []
            changed = False
            for inst in bb.instructions:
                si = inst.sync_info
                if si is not None and len(si.on_wait) > 1:
                    waits = list(si.on_wait)
                    for w in waits[:-1]:
                        _uid[0] += 1
                        new_list.append(mybir.InstNoOp(
                            name=f"I-waitsplit-{_uid[0]}",
                            engine=inst.engine,
                            sync_info=mybir.SyncInfo(on_wait=[w], on_update=[]),
                        ))
                    inst.sync_info = mybir.SyncInfo(
                        on_wait=[waits[-1]], on_update=list(si.on_update))
                    changed = True
                new_list.append(inst)
            if changed:
                bb.instructions.clear()
                bb.instructions.extend(new_list)


def build_nc():
    nc = bass.Bass(trn_type="TRN2", target_bir_lowering=False, debug=False,
                   num_devices=N_CORES)
    xT = nc.dram_tensor("xT", [D, T], bf16, kind="ExternalInput").ap()
    WqT = nc.dram_tensor("WqT", [D, F], bf16, kind="ExternalInput").ap()
    WkT = nc.dram_tensor("WkT", [D, F], bf16, kind="ExternalInput").ap()
    WvT = nc.dram_tensor("WvT", [D, F], bf16, kind="ExternalInput").ap()
    WoT = nc.dram_tensor("WoT", [F, D], bf16, kind="ExternalInput").ap()
    Z = nc.dram_tensor("Z", [T, D], bf16, kind="ExternalOutput").ap()

    xTr = xT.rearrange("(ko p) t -> p ko t", p=P)
    w_r = {
        "q": WqT.rearrange("(ko p) f -> p ko f", p=P),
        "k": WkT.rearrange("(ko p) f -> p ko f", p=P),
        "v": WvT.rearrange("(ko p) f -> p ko f", p=P),
    }

    with tile.TileContext(nc) as tc:
        with (
            tc.tile_pool(name="cw", bufs=1) as cw,
            tc.tile_pool(name="sb1", bufs=1) as sb1,
            tc.tile_pool(name="tp", bufs=4) as tp,
            tc.tile_pool(name="psS", bufs=2, space="PSUM") as psS,
            tc.tile_pool(name="psW", bufs=2, space="PSUM") as psW,
            tc.tile_pool(name="psO", bufs=2, space="PSUM") as psO,
        ):
            # ---- PE warm-up: matmuls on a zeroed tile while DMAs land ----
            zero512 = cw.tile([P, NT], bf16, tag="zero", name="zero512")
            nc.vector.memset(zero512[:], 0.0)
            for r in range(N_WARM // 4):
                for b in range(2):
                    wps = psS.tile([P, 2, NT], f32, tag="s2",
                                   name=f"warm{r}_{b}")
                    for half in range(2):
                        nc.tensor.matmul(wps[:, half], zero512[:, 0:P],
                                         zero512[:], start=True, stop=True)

            # ---- persistent constants / staging ----
            w_sb = {}
            for name in ("q", "k", "v"):
                w_sb[name] = sb1.tile([P, KO, F], bf16, tag=f"w{name}",
                                      name=f"w{name}")
            xt = sb1.tile([P, KO, T], bf16, tag="xt", name="xt")
            # issue order: earliest-needed first (wq+xt@qj0 gate the 1st
            # matmul).  Few LARGE transfers: each dma_start trigger costs
            # ~640ns on the issuing engine, so per-(ko) DMAs serialize the
            # whole input stream behind ~40 triggers.
            nc.sync.dma_start(w_sb["q"][:, 0:2], w_r["q"][:, 0:2])
            nc.sync.dma_start(xt[:, 0:2, 0:NT], xTr[:, 0:2, 0:NT])
            nc.sync.dma_start(w_sb["q"][:, 2:4], w_r["q"][:, 2:4])
            nc.sync.dma_start(xt[:, 2:4, 0:NT], xTr[:, 2:4, 0:NT])
            nc.sync.dma_start(w_sb["q"][:, 4:8], w_r["q"][:, 4:8])
            nc.sync.dma_start(xt[:, 4:8, 0:NT], xTr[:, 4:8, 0:NT])
            nc.sync.dma_start(w_sb["k"][:, 0:4], w_r["k"][:, 0:4])
            nc.sync.dma_start(w_sb["k"][:, 4:8], w_r["k"][:, 4:8])
            nc.sync.dma_start(w_sb["v"][:, 0:4], w_r["v"][:, 0:4])
            nc.sync.dma_start(w_sb["v"][:, 4:8], w_r["v"][:, 4:8])
            for qj in range(1, QJ):
                nc.sync.dma_start(xt[:, :, qj * NT:(qj + 1) * NT],
                                  xTr[:, :, qj * NT:(qj + 1) * NT])

            wo = cw.tile([P, F // P, D], bf16, tag="wo", name="wo")
            nc.gpsimd.dma_start(wo[:], WoT.rearrange("(fo p) d -> p fo d", p=P))
            # allowed[k_row, q_col] = q >= k (upper-triangular incl. diagonal)
            tri = cw.tile([P, P], bf16, tag="tri", name="tri")
            make_upper_triangular(nc, tri[:], val=1.0, diag=True)
            ident = cw.tile([P, P], bf16, tag="ident", name="ident")
            make_identity(nc, ident[:])
            ones_r = cw.tile([1, HD], bf16, tag="ones", name="ones")
            nc.gpsimd.memset(ones_r[:], 1.0)

            # Q/K packed 2 heads per 128 rows: rows 0:64 head 2fs, 64:128
            # head 2fs+1.  The S matmuls contract K=64 per head; the pair
            # runs concurrently in PE row groups (tile_position (0,0)/(64,0)
            # auto-derived from base partitions).
            qt = cw.tile([P, F // P, T], bf16, tag="qt", name="qt")
            kt = cw.tile([P, F // P, T], bf16, tag="kt", name="kt")

            # V with a ones column per head: [k-token, kt, head, 0:64]=V^T,
            # [..., 64]=1 (gives softmax denominators for free in P@V)
            vaug = cw.tile([P, KT, H_LOCAL, HD + 1], bf16, tag="vaug",
                           name="vaug")
            nc.gpsimd.memset(vaug[:, :, :, HD:HD + 1], 1.0)

            ot = cw.tile([P, F // P, T], bf16, tag="ot", name="ot")

            def phase1_units(qj):
                # filler units (~8 PE matmuls each) spliced into attention
                # so the PE stream never drains
                sl = slice(qj * NT, (qj + 1) * NT)
                vt = tp.tile([P, F // P, NT], bf16, tag="vt", bufs=2,
                             name=f"vt{qj}")
                units = []

                def proj(name, fs):
                    def emit():
                        ps = psW.tile([P, NT], f32, tag="w",
                                      name=f"ps_{name}{fs}_{qj}")
                        for ko in range(KO):
                            nc.tensor.matmul(
                                ps[:], w_sb[name][:, ko, fs * P:(fs + 1) * P],
                                xt[:, ko, sl],
                                start=(ko == 0), stop=(ko == KO - 1))
                        if name == "q":
                            nc.vector.tensor_copy(qt[:, fs, sl], ps[:])
                        elif name == "k":
                            nc.vector.tensor_copy(kt[:, fs, sl], ps[:])
                        else:
                            nc.vector.tensor_copy(vt[:, fs, :], ps[:])
                    return emit

                def vtrans(fs):
                    def emit():
                        pst = psW.tile([P, 4, P], bf16, tag="w",
                                       name=f"pvt{fs}_{qj}")
                        for j in range(4):
                            nc.tensor.transpose(
                                pst[:, j], vt[:, fs, j * P:(j + 1) * P],
                                ident[:])
                        nc.vector.tensor_copy(
                            vaug[:, 4 * qj:4 * qj + 4, 2 * fs:2 * fs + 2,
                                 0:HD],
                            pst[:].rearrange("p j (a b) -> p j a b", a=2))
                    return emit

                for name in ("q", "k", "v"):
                    for fs in range(F // P):
                        units.append(proj(name, fs))
                for fs in range(F // P):
                    units.append(vtrans(fs))
                return units

            # after the last attention pair the 4 psS banks are free:
            # alternating tail pz tiles between psW and psS doubles the
            # effective ring depth, so the drain-tail phase4 chain is not
            # serialized on each cast two slots back
            tailpz = [False, 0]
            # while the final division's lane-starved Ln/Exp reciprocals
            # drain on Scalar, filler casts must not queue ahead of them
            resv_dve = [False]

            def phase4_units(qj, tail=False):
                units = []

                def unit(qt_i, dt):
                    def emit():
                        tailpz[1] += 1
                        if tailpz[0] and tailpz[1] % 2 == 0:
                            pz = psS.tile([P, NT], f32, tag="s2",
                                          name=f"pz{qt_i}_{dt}")
                        else:
                            pz = psW.tile([P, NT], f32, tag="w",
                                          name=f"pz{qt_i}_{dt}")
                        for fs in range(F // P):
                            nc.tensor.matmul(
                                pz[:], ot[:, fs, qt_i * P:(qt_i + 1) * P],
                                wo[:, fs, dt * NT:(dt + 1) * NT],
                                start=(fs == 0), stop=(fs == F // P - 1))
                        zs = tp.tile([P, NT], bf16, tag="z", bufs=8,
                                     name=f"zs{qt_i}_{dt}")
                        if dt == 1 and not resv_dve[0]:
                            # alternate the PSUM->SBUF casts across both
                            # engines: halves the psW WAR chain latency;
                            # Scalar has enough bubbles even while the exp
                            # stream runs, and is idle in the drain tail
                            nc.scalar.activation(
                                zs[:], pz[:],
                                mybir.ActivationFunctionType.Copy)
                        else:
                            nc.vector.tensor_copy(zs[:], pz[:])
                        # alternate trigger queues so the output stream is
                        # not serialized behind one engine's ~640ns triggers
                        # (in the tail, keep everything on sync so gpsimd's
                        # software DMA queue can drain early)
                        if tail:
                            eng = nc.sync if (qt_i + dt) % 2 == 0 \
                                else nc.scalar
                        else:
                            eng = nc.sync if (qt_i + dt) % 2 == 0 \
                                else nc.gpsimd
                        eng.dma_start(
                            Z[qt_i * P:(qt_i + 1) * P,
                              dt * NT:(dt + 1) * NT], zs[:])
                    return emit

                for qt_i in range(4 * qj, 4 * qj + 4):
                    for dt in range(D // NT):
                        units.append(unit(qt_i, dt))
                return units

            def attn_pair(p, qj, pending, take_filler):
                # heads 2p (rows 0:64) and 2p+1 (rows 64:128) of group fs=p
                n_ki = 4 * qj + 4
                po0 = psO.tile([HD + 1, NT], f32, tag="o",
                               name=f"po{p}_{qj}_0")
                po1 = psO.tile([HD + 1, NT], f32, tag="o",
                               name=f"po{p}_{qj}_1")
                pts = {}

                def s_pair(ki):
                    col0 = 0 if ki < 4 * qj else (ki - 4 * qj) * P
                    N = NT - col0
                    ps = psS.tile([P, 2, NT], f32, tag="s2",
                                  name=f"pss{p}_{qj}_{ki}")
                    qs = slice(qj * NT + col0, (qj + 1) * NT)
                    ks = slice(ki * P, (ki + 1) * P)
                    nc.tensor.matmul(ps[:, 0, 0:N], kt[0:HD, p, ks],
                                     qt[0:HD, p, qs], start=True, stop=True)
                    nc.tensor.matmul(ps[:, 1, 0:N], kt[HD:P, p, ks],
                                     qt[HD:P, p, qs], start=True, stop=True)
                    pt = tp.tile([P, 2, NT], bf16, tag="pt", bufs=LAG + 1,
                                 name=f"pt{p}_{qj}_{ki}")
                    # one exp over both heads' banks (h0 cols N:512 are
                    # unused garbage, exp'd harmlessly)
                    w = NT + N
                    nc.scalar.activation(
                        pt[:].rearrange("p a n -> p (a n)")[:, 0:w],
                        ps[:].rearrange("p a n -> p (a n)")[:, 0:w],
                        mybir.ActivationFunctionType.Exp, scale=0.125)
                    if ki >= 4 * qj:
                        nc.gpsimd.tensor_mul(pt[:, 0, 0:P], pt[:, 0, 0:P],
                                             tri[:])
                        nc.gpsimd.tensor_mul(pt[:, 1, 0:P], pt[:, 1, 0:P],
                                             tri[:])
                    pts[ki] = (pt, col0, N)

                def o_pair(ki):
                    pt, col0, N = pts.pop(ki)
                    st = (ki == 0)
                    sp = (ki == n_ki - 1)
                    nc.tensor.matmul(po0[:, col0:NT], vaug[:, ki, 2 * p, :],
                                     pt[:, 0, 0:N], start=st, stop=sp)
                    nc.tensor.matmul(po1[:, col0:NT], vaug[:, ki, 2 * p + 1, :],
                                     pt[:, 1, 0:N], start=st, stop=sp)

                # the sibling pair's division runs deep into this pair so
                # its PE broadcasts never wait on the (slow, lane-starved)
                # reciprocals
                div_step = min(LAG + 6, n_ki + LAG - 1)
                for step in range(n_ki + LAG):
                    if step < n_ki:
                        s_pair(step)
                    if step == div_step and pending is not None:
                        pending()
                        pending = None
                    if step >= LAG:
                        o_pair(step - LAG)
                    if step >= 2:
                        # filler from step 2 on: steps 0-1 prime the exp
                        # pipe; 2..LAG-1 would otherwise stall on the S
                        # PSUM ring while the exp stream catches up
                        take_filler()

                # reciprocals issued now (DVE), division deferred so the PE
                # broadcast matmuls queue behind later attention work
                # so copies FIRST: they are the last po readers, and the
                # next pair's P@V reuses these PSUM banks (WAR) — putting
                # them behind a slow reciprocal stalls the PE there.
                sos = []
                for h, po in ((0, po0), (1, po1)):
                    so = tp.tile([HD, NT], f32, tag="so", bufs=4,
                                 name=f"so{p}_{qj}_{h}")
                    nc.vector.tensor_copy(so[:], po[0:HD, :])
                    sos.append(so)
                # Reciprocal as exp(-ln d) on Scalar: [1, N] ops are
                # lane-starved everywhere, but Scalar's Ln/Exp pair
                # (~1.2us) beats DVE's reciprocal (~3.3us), and keeping
                # them off DVE keeps its queue shallow for the so copies
                # that release the po banks (next pair's P@V WARs).
                rrs = []
                for h, po in ((0, po0), (1, po1)):
                    rrt = tp.tile([1, NT], bf16, tag="rr", bufs=4,
                                  name=f"rr{p}_{qj}_{h}")
                    lnt = tp.tile([1, NT], f32, tag="ln", bufs=4,
                                  name=f"ln{p}_{qj}_{h}")
                    nc.scalar.activation(lnt[:], po[HD:HD + 1, :],
                                         mybir.ActivationFunctionType.Ln)
                    nc.scalar.activation(rrt[:], lnt[:],
                                         mybir.ActivationFunctionType.Exp,
                                         scale=-1.0)
                    rrs.append((rrt, sos[h]))

                def division():
                    for h, (rrt, so) in enumerate(rrs):
                        pb = psW.tile([HD, NT], f32, tag="w",
                                      name=f"pb{p}_{qj}_{h}")
                        nc.tensor.matmul(pb[:], ones_r[:], rrt[:],
                                         start=True, stop=True)
                        nc.vector.tensor_mul(
                            ot[HD * h:HD * (h + 1), p,
                               qj * NT:(qj + 1) * NT],
                            so[:], pb[:])
                return division

            # ---- schedule ----
            for u in phase1_units(0):
                u()
            backlog = []
            for qj in range(QJ):
                # phase1(qj+1) must complete within this qj (its attention
                # needs it), but phase4 units are movable: hold them for the
                # last, largest qj, whose exp stream otherwise saturates
                # Scalar while the PE runs filler-dry (and HAM re-throttles
                # the PE clock below ~90% activity).
                filler = list(phase1_units(qj + 1)) if qj + 1 < QJ else []
                if qj == QJ - 1:
                    filler += backlog
                    backlog = []
                # on the last qj, hold back a few units: they cover the PE
                # while the final division chain (lane-starved scalar recip
                # -> PE broadcast -> DVE mul) drains before the tail phase4
                spare = 4 if qj == QJ - 1 else 3
                reserved = filler[len(filler) - spare:]
                filler = filler[:len(filler) - spare]
                n_points = 2 * (4 * qj + 4 + LAG - 2)
                state = [0, 0]  # points passed, units taken

                def take_filler(filler=filler, state=state,
                                n_points=n_points):
                    state[0] += 1
                    want = (len(filler) * state[0]) // n_points
                    while state[1] < want:
                        filler[state[1]]()
                        state[1] += 1

                div0 = attn_pair(0, qj, None, take_filler)
                div1 = attn_pair(1, qj, div0, take_filler)
                if qj == QJ - 1:
                    tailpz[0] = True
                    resv_dve[0] = True
                while state[1] < len(filler):
                    filler[state[1]]()
                    state[1] += 1
                for u in reserved:
                    u()
                div1()
                resv_dve[0] = False
                backlog += phase4_units(qj, tail=(qj == QJ - 1))
            for u in backlog:
                u()

    _legalize_single_wait(nc)
    return nc


def _make_in_maps(x, Wq, Wk, Wv, Wo):
    bf = ml_dtypes.bfloat16
    in_maps = []
    for c in range(N_CORES):
        b, g = divmod(c, 4)
        sl = slice(g * F, (g + 1) * F)
        in_maps.append({
            "xT": np.ascontiguousarray(np.asarray(x)[b].T).astype(bf),
            "WqT": np.ascontiguousarray(np.asarray(Wq)[sl, :].T).astype(bf),
            "WkT": np.ascontiguousarray(np.asarray(Wk)[sl, :].T).astype(bf),
            "WvT": np.ascontiguousarray(np.asarray(Wv)[sl, :].T).astype(bf),
            "WoT": np.ascontiguousarray(np.asarray(Wo)[:, sl].T).astype(bf),
        })
    return in_maps


def run(x, Wq, Wk, Wv, Wo, trace=False, trace_cores=None):
    nc = build_nc()
    in_maps = _make_in_maps(x, Wq, Wk, Wv, Wo)
    res = run_bass_kernel_spmd(nc, in_maps, list(range(N_CORES)), trace=trace,
                               trace_cores=trace_cores)
    out = np.zeros((B, T, D), np.float32)
    for c in range(N_CORES):
        out[c // 4] += res.results[c]["Z"].astype(np.float32)
    return out, res


def kernel(x, Wq, Wk, Wv, Wo):
    try:
        out, _ = run(x, Wq, Wk, Wv, Wo)
    except Exception:
        # one retry for transient device errors (e.g. a wedged core from a
        # prior run)
        out, _ = run(x, Wq, Wk, Wv, Wo)
    return out
